# revision 1
# baseline (speedup 1.0000x reference)
"""Cascaded group attention (EfficientViT-style) on 8 Trainium2 NeuronCores.

Data-parallel over batch: 1024 items -> 128 per core. Per core the head
cascade runs serially; within a head, batch is processed in groups so the
depthwise-conv spatial matmuls and the qkv matmuls amortize stationary
weights across the group.

Layout strategy per head (per batch item b, N=196 spatial positions):
  feat   [c=65 | n]      (row 64 is constant 1.0 so qkv biases fold into
                          the matmul as an extra contraction row)
  k      [c=16 | m]      4 items packed at 32-partition offsets (PE column
                          tiling -> concurrent matmuls)
  q~T,vT [n | c]         from feat-as-stationary matmuls (transposed
                          outputs feed the spatial depthwise conv and the
                          attention-value product without extra transposes)
  qdwT   [n | b,c]       5x5 depthwise conv as dense [196,196] spatial
                          matmul vs the (zero-padded-at-boundary) conv
                          matrix, batched over the group in the free dim
  qdw    [c | n]         PE transpose, 4 items col-packed per instruction
  attnT  [m | n]         scores matmul; softmax runs over partitions, so
                          normalization is deferred: exp on ACT, the
                          relative-position bias enters as a separate
                          elementwise multiply exp(qk)*exp(ab), the
                          denominator Z comes from a ones-column in the
                          attention-value stationary, and the divide is a
                          partition-broadcast (GPSIMD) + tensor divide.
  av     [d=64+1 | n]    value product; row 64 = Z
  proj               c=256 contraction in two 128-row chunks, relu'd head
                     outputs gathered in two [128 | G*196] tiles
"""

import sys

sys.path.insert(0, "/opt/trn_rl_repo")

import numpy as np

NUM_HEADS = 4
KEY_DIM = 16
DIM = 256
D = 64
RES = 14
N = RES * RES  # 196
NH = 98  # N // 2, partition chunk
KER = 5
NCORES = 8
B = 1024

_cache = {}


def _patch_tile_drain():
    """The TileContext tail drain can accumulate >2 sem waits; the CoreV3
    CTRL encoding rejects that. Split extra waits across additional
    single-wait drain instructions."""
    import concourse.tile as tile
    from concourse.vector_clock import ScopedClock
    from concourse import mybir

    if getattr(tile.TileContext, "_drain_split_patched", False):
        return

    def _split_drain_and_barrier(self, tick_clock, wait_clock):
        nc = self.nc
        drain_inst = nc.sync.drain()
        wait_clock.add_sem_waits(
            drain_inst.ins, ScopedClock({None: tick_clock.global_clock})
        )
        si = drain_inst.ins.sync_info
        if si is not None and si.on_wait and len(si.on_wait) > 1:
            waits = list(si.on_wait)
            si.on_wait.clear()
            si.on_wait.append(waits[0])
            for w in waits[1:]:
                d2 = nc.sync.drain()
                s2 = d2.ins.sync_info
                if s2 is None:
                    d2.ins.sync_info = mybir.SyncInfo(on_wait=[w], on_update=[])
                else:
                    s2.on_wait.append(w)
        nc.all_engine_barrier()
        assert self.sems is not None
        popped = nc._tile_sem_poison_stack.pop()
        assert popped is self._sem_poison
        nc.clear_and_free_semaphores(list(self.sems.allocated().values()))
        nc.all_engine_barrier()

    tile.TileContext._drain_and_barrier = _split_drain_and_barrier
    tile.TileContext._drain_split_patched = True


def build_bass(b_core, g_sz):
    """Emit the per-core kernel. b_core batch items per core, processed in
    groups of g_sz (g_sz % 4 == 0)."""
    import concourse.bass as bass
    import concourse.tile as tile
    from concourse import mybir
    from concourse.masks import make_identity

    _patch_tile_drain()

    assert b_core % g_sz == 0 and g_sz % 4 == 0
    n_groups = b_core // g_sz
    G = g_sz
    f32 = mybir.dt.float32

    nc = bass.Bass()
    xp = nc.declare_dram_parameter("x", [b_core, DIM, N], f32, isOutput=False)
    wk_p = nc.declare_dram_parameter("wk65", [65, NUM_HEADS, 32], f32, isOutput=False)
    wqv_p = nc.declare_dram_parameter("wqv65", [65, NUM_HEADS, 80], f32, isOutput=False)
    dt_p = nc.declare_dram_parameter("dt", [NUM_HEADS, 16, N, N], f32, isOutput=False)
    dwb_p = nc.declare_dram_parameter("dwb", [64], f32, isOutput=False)
    e_p = nc.declare_dram_parameter("e_t", [NUM_HEADS, 2, NH, N], f32, isOutput=False)
    wpj_p = nc.declare_dram_parameter("wprojT", [2, 2, 128, 128], f32, isOutput=False)
    pjb_p = nc.declare_dram_parameter("projb", [2, 128], f32, isOutput=False)
    ones_p = nc.declare_dram_parameter("ones_g", [1, G * N], f32, isOutput=False)
    out_p = nc.declare_dram_parameter("out", [b_core, DIM, N], f32, isOutput=True)

    with tile.TileContext(nc) as tc:
        _emit(nc, tc, locals())
    _split_excess_waits(nc)
    return nc


def _split_excess_waits(nc, maxw=1):
    """This neuronxcc rejects >2 sem waits per instruction. Hoist extras onto
    NoOp instructions inserted immediately before, on the same engine."""
    from concourse import mybir

    for _, blk in nc.bb_map.items():
        il = blk.bb.instructions
        idx = 0
        while idx < len(il):
            inst = il[idx]
            si = getattr(inst, "sync_info", None)
            if si is not None and si.on_wait and len(si.on_wait) > maxw:
                waits = list(si.on_wait)
                si.on_wait.clear()
                for w in waits[:maxw]:
                    si.on_wait.append(w)
                extra = waits[maxw:]
                pos = idx
                for j in range(0, len(extra), maxw):
                    nop = mybir.InstNoOp(name=f"{inst.name}-ws{j}", ins=[], outs=[])
                    nop.engine = inst.engine
                    nop.sync_info = mybir.SyncInfo(
                        on_wait=extra[j : j + maxw], on_update=[]
                    )
                    il.insert(pos, nop)
                    pos += 1
                    idx += 1
            idx += 1


def _emit(nc, tc, P):
    import concourse.bass as bass
    from concourse import mybir
    from concourse.masks import make_identity
    from contextlib import ExitStack

    f32 = mybir.dt.float32
    G = P["G"]
    n_groups = P["n_groups"]
    xp, out_p = P["xp"], P["out_p"]
    Exp = mybir.ActivationFunctionType.Exp
    AluOp = mybir.AluOpType

    with ExitStack() as ctx:
        cpool = ctx.enter_context(tc.tile_pool(name="consts", bufs=1))
        featpool = ctx.enter_context(tc.tile_pool(name="feat", bufs=2))
        kgpool = ctx.enter_context(tc.tile_pool(name="kg", bufs=1))
        qtpool = ctx.enter_context(tc.tile_pool(name="qt", bufs=1))
        vtpool = ctx.enter_context(tc.tile_pool(name="vt", bufs=1))
        qdtpool = ctx.enter_context(tc.tile_pool(name="qdwT", bufs=1))
        qdwpool = ctx.enter_context(tc.tile_pool(name="qdw", bufs=1))
        dtpool = ctx.enter_context(tc.tile_pool(name="dt", bufs=2))
        attsbpool = ctx.enter_context(tc.tile_pool(name="attsb", bufs=2))
        avpool = ctx.enter_context(tc.tile_pool(name="avsb", bufs=4))
        catpool = ctx.enter_context(tc.tile_pool(name="cat", bufs=1))
        poutpool = ctx.enter_context(tc.tile_pool(name="pout", bufs=4))
        zdpool = ctx.enter_context(tc.tile_pool(name="zd", bufs=6, space="DRAM"))

        kqps = ctx.enter_context(tc.tile_pool(name="kqps", bufs=1, space="PSUM"))
        qvps = ctx.enter_context(tc.tile_pool(name="qvps", bufs=1, space="PSUM"))
        dwps = ctx.enter_context(tc.tile_pool(name="dwps", bufs=1, space="PSUM"))
        tpps = ctx.enter_context(tc.tile_pool(name="tpps", bufs=1, space="PSUM"))
        attps = ctx.enter_context(tc.tile_pool(name="attps", bufs=1, space="PSUM"))
        avps = ctx.enter_context(tc.tile_pool(name="avps", bufs=1, space="PSUM"))
        pjps = ctx.enter_context(tc.tile_pool(name="pjps", bufs=1, space="PSUM"))

        # ---- constants ----
        wk = cpool.tile([65, NUM_HEADS, 32], f32)
        nc.sync.dma_start(out=wk[:], in_=P["wk_p"][:])
        wqv = cpool.tile([65, NUM_HEADS, 80], f32)
        nc.sync.dma_start(out=wqv[:], in_=P["wqv_p"][:])
        dwb = cpool.tile([NH, 64], f32)
        dwb_ap = P["dwb_p"][:]
        nc.sync.dma_start(
            out=dwb[:],
            in_=bass.AP(tensor=dwb_ap.tensor, offset=dwb_ap.offset, ap=[[0, NH], [1, 64]]),
        )
        e_sb = cpool.tile([NH, NUM_HEADS, 2, N], f32)
        for h in range(NUM_HEADS):
            for mp in range(2):
                nc.sync.dma_start(out=e_sb[:, h, mp, :], in_=P["e_p"][h, mp, :, :])
        wpj = cpool.tile([128, 2, 2, 128], f32)
        for kc in range(2):
            for mo in range(2):
                nc.sync.dma_start(out=wpj[:, kc, mo, :], in_=P["wpj_p"][kc, mo, :, :])
        pjb = cpool.tile([128, 2], f32)
        pjb_ap = P["pjb_p"][:]
        for mo in range(2):
            nc.sync.dma_start(
                out=pjb[:, mo : mo + 1],
                in_=bass.AP(
                    tensor=pjb_ap.tensor,
                    offset=pjb_ap.offset + mo * 128,
                    ap=[[1, 128], [0, 1]],
                ),
            )
        ident = cpool.tile([128, 128], f32)
        make_identity(nc, ident)

        # ---- main loop ----
        for g in range(n_groups):
            b0 = g * G
            feat = featpool.tile([65, G, N], f32, tag="feat")
            nc.sync.dma_start(
                out=feat[0:64, :, :],
                in_=xp[b0 : b0 + G, 0:64, :].transpose([1, 0, 2]),
            )
            nc.sync.dma_start(
                out=feat[64:65, :, :].rearrange("p a b -> p (a b)"), in_=P["ones_p"][:]
            )
            cat = [catpool.tile([128, G, N], f32, tag=f"cat{kc}", name=f"cat{kc}") for kc in range(2)]

            for h in range(NUM_HEADS):
                # prefetch next head's x slice
                if h < NUM_HEADS - 1:
                    fnext = featpool.tile([65, G, N], f32, tag="feat")
                    nc.sync.dma_start(
                        out=fnext[0:64, :, :],
                        in_=xp[b0 : b0 + G, (h + 1) * 64 : (h + 2) * 64, :].transpose(
                            [1, 0, 2]
                        ),
                    )
                    nc.sync.dma_start(
                        out=fnext[64:65, :, :].rearrange("p a b -> p (a b)"),
                        in_=P["ones_p"][:],
                    )
                else:
                    fnext = None

                # --- phase A: k + (q~T, vT) ---
                kg = kgpool.tile([128, G // 4, N], f32, tag="kg")
                qt = [qtpool.tile([NH, G, 32], f32, tag=f"qt{mp}", name=f"qt{mp}") for mp in range(2)]
                vt = [vtpool.tile([NH, G, 65], f32, tag=f"vt{mp}", name=f"vt{mp}") for mp in range(2)]
                for mp in range(2):
                    nc.vector.memset(vt[mp][:, :, 64:65], 1.0)
                for s4 in range(G // 4):
                    kq = kqps.tile([128, 512], f32, tag="kq")
                    for sb in range(4):
                        b = s4 * 4 + sb
                        nc.tensor.matmul(
                            kq[32 * sb : 32 * sb + 32, 0:N],
                            wk[:, h, :],
                            feat[:, b, :],
                            start=True,
                            stop=True,
                            tile_position=(0, 32 * sb),
                        )
                    nc.vector.tensor_copy(kg[:, s4, :], kq[:, 0:N])
                    for mp in range(2):
                        qv = qvps.tile([NH, 4, 128], f32, tag="qv", name="qv")
                        for sb in range(4):
                            b = s4 * 4 + sb
                            nc.tensor.matmul(
                                qv[:, sb, 0:80],
                                feat[:, b, mp * NH : (mp + 1) * NH],
                                wqv[:, h, :],
                                start=True,
                                stop=True,
                            )
                        nc.vector.tensor_copy(
                            qt[mp][:, s4 * 4 : s4 * 4 + 4, 0:16], qv[:, :, 0:16]
                        )
                        nc.vector.tensor_copy(
                            vt[mp][:, s4 * 4 : s4 * 4 + 4, 0:64], qv[:, :, 16:80]
                        )

                # --- phase B: depthwise conv as spatial matmul ---
                qdwT = [
                    qdtpool.tile([NH, G, 32], f32, tag=f"qdwT{mp}", name=f"qdwT{mp}") for mp in range(2)
                ]
                for mp in range(2):
                    nc.vector.memset(qdwT[mp][:, :, 16:32], 0.0)
                for c in range(16):
                    dtt = [dtpool.tile([NH, N], f32, tag="dt", name="dtt") for _ in range(2)]
                    for kp in range(2):
                        nc.sync.dma_start(
                            out=dtt[kp][:],
                            in_=P["dt_p"][h, c, kp * NH : (kp + 1) * NH, :],
                        )
                    for mp in range(2):
                        dw = dwps.tile([NH, 512], f32, tag="dw")
                        for kp in range(2):
                            nc.tensor.matmul(
                                dw[:, 0:G],
                                dtt[kp][:, mp * NH : (mp + 1) * NH],
                                qt[kp][:, :, c],
                                start=(kp == 0),
                                stop=(kp == 1),
                            )
                        nc.vector.tensor_scalar_add(
                            qdwT[mp][:, :, c], dw[:, 0:G], dwb[:, h * 16 + c : h * 16 + c + 1]
                        )

                # --- phase C: transpose qdwT -> qdw [c|n], 4 items col-packed ---
                qdw = qdwpool.tile([128, G // 4, N], f32, tag="qdw")
                for s4 in range(G // 4):
                    for mp in range(2):
                        tp = tpps.tile([128, 512], f32, tag="tp")
                        nc.tensor.transpose(
                            tp[:, 0:NH],
                            qdwT[mp][:, s4 * 4 : s4 * 4 + 4, :].rearrange(
                                "p a b -> p (a b)"
                            ),
                            ident[0:NH, 0:NH],
                        )
                        nc.vector.tensor_copy(qdw[:, s4, mp * NH : (mp + 1) * NH], tp[:, 0:NH])

                # --- phase D: attention ---
                for s2 in range(G // 2):
                    att = attps.tile([NH, 2, 512], f32, tag="att")
                    for j in range(2):
                        b = s2 * 2 + j
                        sb = b % 4
                        s4 = b // 4
                        for mp in range(2):
                            nc.tensor.matmul(
                                att[:, j, mp * N : mp * N + N],
                                kg[32 * sb : 32 * sb + 16, s4, mp * NH : (mp + 1) * NH],
                                qdw[32 * sb : 32 * sb + 16, s4, :],
                                start=True,
                                stop=True,
                                tile_position=(32 * sb, 0),
                            )
                    attsb = attsbpool.tile([NH, 2, 2, N], f32, tag="attsb")
                    nc.scalar.activation(
                        attsb[:].rearrange("p a b c -> p a (b c)"),
                        att[:, :, 0 : 2 * N],
                        Exp,
                    )
                    for j in range(2):
                        nc.vector.tensor_tensor(
                            attsb[:, j, :, :],
                            attsb[:, j, :, :],
                            e_sb[:, h, :, :],
                            AluOp.mult,
                        )
                    for j in range(2):
                        b = s2 * 2 + j
                        av = avps.tile([65, 512], f32, tag="av")
                        for kp in range(2):
                            nc.tensor.matmul(
                                av[:, 0:N],
                                vt[kp][:, b, :],
                                attsb[:, j, kp, :],
                                start=(kp == 0),
                                stop=(kp == 1),
                            )
                        avsb = avpool.tile([64, N], f32, tag="avsb")
                        nc.vector.tensor_copy(avsb[:], av[0:64, 0:N])
                        rrow = avpool.tile([1, N], f32, tag="rrow")
                        nc.vector.reciprocal(rrow[:], av[64:65, 0:N])
                        zd = zdpool.tile([1, N], f32, tag="zd", name="zd")
                        nc.gpsimd.dma_start(out=zd[:], in_=rrow[:])
                        zb = avpool.tile([64, N], f32, tag="zb")
                        zd_ap = zd[:]
                        nc.gpsimd.dma_start(
                            out=zb[:],
                            in_=bass.AP(
                                tensor=zd_ap.tensor,
                                offset=zd_ap.offset,
                                ap=[[0, 64], [1, N]],
                            ),
                        )
                        outn = avpool.tile([64, N], f32, tag="outn")
                        nc.vector.tensor_tensor(
                            outn[:], avsb[:], zb[:], AluOp.mult
                        )
                        nc.gpsimd.tensor_scalar_max(
                            cat[h // 2][(h % 2) * 64 : (h % 2) * 64 + 64, b, :],
                            outn[:],
                            0.0,
                        )
                        if fnext is not None:
                            nc.vector.tensor_tensor(
                                fnext[0:64, b, :], outn[:], fnext[0:64, b, :], AluOp.add
                            )
                feat = fnext

            # --- proj ---
            for b in range(G):
                for mo in range(2):
                    pj = pjps.tile([128, 512], f32, tag="pj")
                    for kc in range(2):
                        nc.tensor.matmul(
                            pj[:, 0:N],
                            wpj[:, kc, mo, :],
                            cat[kc][:, b, :],
                            start=(kc == 0),
                            stop=(kc == 1),
                        )
                    po = poutpool.tile([128, N], f32, tag="po")
                    nc.vector.tensor_scalar_add(po[:], pj[:, 0:N], pjb[:, mo : mo + 1])
                    nc.gpsimd.dma_start(
                        out=out_p[b0 + b, mo * 128 : (mo + 1) * 128, :], in_=po[:]
                    )


def host_prep(qkv_w, qkv_b, dw_w, dw_b, proj_w, proj_b, attention_biases, bias_idxs, g_sz):
    """Precompute device-friendly weight layouts on the host."""
    qkv_w = np.asarray(qkv_w, np.float32)
    qkv_b = np.asarray(qkv_b, np.float32)
    dw_w = np.asarray(dw_w, np.float32)
    dw_b = np.asarray(dw_b, np.float32)
    proj_w = np.asarray(proj_w, np.float32)
    proj_b = np.asarray(proj_b, np.float32)
    ab = np.asarray(attention_biases, np.float32)[:, np.asarray(bias_idxs)]  # [4,N,N]

    scale = KEY_DIM ** (-0.5)

    wk65 = np.zeros([65, NUM_HEADS, 32], np.float32)
    wqv65 = np.zeros([65, NUM_HEADS, 80], np.float32)
    for h in range(NUM_HEADS):
        wk65[0:64, h, 0:16] = qkv_w[h, 16:32, :].T
        wk65[64, h, 0:16] = qkv_b[h, 16:32]
        wqv65[0:64, h, 0:16] = qkv_w[h, 0:16, :].T
        wqv65[0:64, h, 16:80] = qkv_w[h, 32:96, :].T
        wqv65[64, h, 0:16] = qkv_b[h, 0:16]
        wqv65[64, h, 16:80] = qkv_b[h, 32:96]

    # depthwise conv as dense spatial matrix, transposed for the lhsT slot:
    # dt[h,c,n',n] = scale * w[h,c, y'-y+2, x'-x+2]  (n=(y,x) output pos)
    yy, xx = np.divmod(np.arange(N), RES)
    dy = yy[None, :] - yy[:, None]  # [n, n'] = y' - y
    dx = xx[None, :] - xx[:, None]
    valid = (np.abs(dy) <= 2) & (np.abs(dx) <= 2)
    ky = np.clip(dy + 2, 0, 4)
    kx = np.clip(dx + 2, 0, 4)
    dt = np.zeros([NUM_HEADS, 16, N, N], np.float32)
    for h in range(NUM_HEADS):
        for c in range(16):
            m = dw_w[h, c][ky, kx] * valid  # [n, n']
            dt[h, c] = (scale * m).T  # [n', n]
    dwb = (scale * dw_b).reshape(64).astype(np.float32)

    e_t = np.zeros([NUM_HEADS, 2, NH, N], np.float32)
    for h in range(NUM_HEADS):
        eh = np.exp(ab[h].T)  # [m, n]
        e_t[h, 0] = eh[0:NH, :]
        e_t[h, 1] = eh[NH:, :]

    wprojT = np.zeros([2, 2, 128, 128], np.float32)
    for kc in range(2):
        for mo in range(2):
            wprojT[kc, mo] = proj_w[mo * 128 : (mo + 1) * 128, kc * 128 : (kc + 1) * 128].T
    projb = proj_b.reshape(2, 128).astype(np.float32)

    return {
        "wk65": wk65,
        "wqv65": wqv65,
        "dt": dt,
        "dwb": dwb,
        "e_t": e_t,
        "wprojT": wprojT,
        "projb": projb,
        "ones_g": np.ones([1, g_sz * N], np.float32),
    }


def run(inputs, b_core, g_sz):
    from concourse.bass_utils import run_bass_kernel_spmd

    key = (b_core, g_sz)
    if key not in _cache:
        _cache[key] = build_bass(b_core, g_sz)
    nc = _cache[key]

    x = np.ascontiguousarray(np.asarray(inputs["x"], np.float32))
    Btot = x.shape[0]
    assert Btot == b_core * NCORES
    shared = host_prep(
        inputs["qkv_w"], inputs["qkv_b"], inputs["dw_w"], inputs["dw_b"],
        inputs["proj_w"], inputs["proj_b"], inputs["attention_biases"],
        inputs["bias_idxs"], g_sz,
    )
    xr = x.reshape(NCORES, b_core, DIM, N)
    in_maps = [dict(x=np.ascontiguousarray(xr[i]), **shared) for i in range(NCORES)]
    res = run_bass_kernel_spmd(nc, in_maps, list(range(NCORES))).results
    out = np.stack([res[i]["out"] for i in range(NCORES)])
    return out.reshape(Btot, DIM, RES, RES)


def kernel(**inputs):
    return run(inputs, b_core=B // NCORES, g_sz=32)

